# revision 3
# baseline (speedup 1.0000x reference)
"""JPEG layer (DCT quantize/dequantize roundtrip) as a Bass/Tile kernel on 8 trn2 cores.

Data parallel: 32 images -> 4 per core. Everything per-image stays in SBUF.

Pipeline per image (channel planes 512x512 f32):
  DMA in [128(wlow), c, wc, h]
  pass1 (PE, f32r, data-as-weights): contract w within 8(16)-blocks, fused color
        transform (XF1) + chroma 2x2 w-pooling. out M1[h-part, (wc,[Y|Cb|Cr])]
  pass2 (PE, f32r, const weights):   contract h. out DCT coeffs [h'-part, w']
  middle (DVE/GPSIMD/ACT): quant = rne((dct - dcshift)/q) via magic-number trick,
        stats (mean|quant|, zero count), decomp = quant*q -> bf16
  pass3 (PE, bf16, data-as-weights): IDCT contract h' (+chroma h-upsample),
        out M3[w'-part, h]
  pass4 (PE, f32r, const weights):   IDCT contract w' (+chroma w-upsample),
        fused XF2 color transform accumulated in PSUM; +HALF on eviction
  DMA out
"""

import math

import ml_dtypes
import numpy as np

import concourse.bass as bass
import concourse.mybir as mybir
import concourse.tile as tile
from concourse import bacc
from concourse.bass_utils import run_bass_kernel_spmd

N_CORES = 8
B_CORE = 4          # images per core
BS = 8
HALF = 128.0 / 255.0
MAGIC = float(1.5 * 2 ** 23)

F32 = mybir.dt.float32
F32R = mybir.dt.float32r
BF16 = mybir.dt.bfloat16
AX = mybir.AxisListType
ALU = mybir.AluOpType
ACTF = mybir.ActivationFunctionType

XF1 = np.array([[0.299, 0.587, 0.114],
                [-0.168735892, -0.331264108, 0.5],
                [0.5, -0.418687589, -0.081312411]], dtype=np.float32)
XF2 = np.array([[1.0, 0.0, 1.402],
                [1.0, -0.344136286, -0.714136286],
                [1.0, 1.772, 0.0]], dtype=np.float32)


def _dctmtx():
    j = np.arange(BS)
    i = np.arange(BS)
    m = np.sqrt(2.0 / BS) * np.cos(np.pi * (2 * j[None, :] + 1) * i[:, None] / (2 * BS))
    m[0, :] = 1.0 / np.sqrt(BS)
    return m.astype(np.float32)


def _blockdiag(block, nblk):
    """block [r, c] -> [nblk*r, nblk*c] block diagonal."""
    r, c = block.shape
    out = np.zeros((nblk * r, nblk * c), dtype=np.float32)
    for b in range(nblk):
        out[b * r:(b + 1) * r, b * c:(b + 1) * c] = block
    return out


def make_consts(qtable: np.ndarray) -> dict[str, np.ndarray]:
    D = _dctmtx()
    # E: pooled DCT row op (8x16), F: IDCT+upsample row op (16x8)
    E = np.zeros((8, 16), dtype=np.float32)
    for u in range(8):
        for j in range(16):
            E[u, j] = 0.5 * D[u, j // 2]
    F = np.zeros((16, 8), dtype=np.float32)
    for r in range(16):
        for v in range(8):
            F[r, v] = D[v, r // 2]

    WY = _blockdiag(D.T, 16)            # [128,128]  w-DCT, used transposed-contract
    WE = _blockdiag(E.T, 8)             # [128,64]   pooled w(or h)-DCT
    rhs1 = np.stack([
        np.concatenate([WY * XF1[0, c], WE * XF1[1, c], WE * XF1[2, c]], axis=1)
        for c in range(3)
    ])                                   # [3,128,256]
    W2h = _blockdiag(D, 16)             # [128,128]  IDCT contract h' (rhs pass3 Y)
    WFh = _blockdiag(F.T, 8)            # [64,128]   IDCT+up contract h'_c
    W2w = _blockdiag(D, 16)             # [128,128]  IDCT contract w' (lhsT pass4 Y)
    WFw64 = _blockdiag(F.T, 8)          # [64,128]
    WFw = np.concatenate([WFw64, WFw64], axis=0)    # [128,128] dup for base 0/64
    wfw = np.stack([
        WFw * XF2[1, 1],   # cb->G
        WFw * XF2[2, 1],   # cb->B
        WFw * XF2[0, 2],   # cr->R
        WFw * XF2[1, 2],   # cr->G
    ])                                   # [4,128,128]

    q = qtable.astype(np.float32)        # q[u, v]
    p = np.arange(128)
    f = np.arange(512)
    qpf = q[np.ix_(f % 8, p % 8)].T      # [128,512]: q[f%8, p%8]
    rq = (1.0 / qpf).astype(np.float32)
    q4 = np.tile(qpf, (1, 4)).astype(np.float32)          # [128,2048]
    cy = np.zeros((128, 512), dtype=np.float32)
    dc = np.ix_(p % 8 == 0, f % 8 == 0)
    cy[dc] = (8.0 * HALF) * rq[dc]
    cy4 = np.tile(cy, (1, 4)).astype(np.float32)          # [128,2048]

    return {
        "rhs1": rhs1,
        "w1h": WY,                       # lhsT pass2 Y (same matrix as WY)
        "we": WE,
        "w2h": W2h.astype(ml_dtypes.bfloat16),
        "wfh": WFh.astype(ml_dtypes.bfloat16),
        "w2w": W2w,
        "wfw": wfw,
        "rq": rq,
        "q4": q4,
        "cy4": cy4,
    }


def build_nc(b_core: int = B_CORE):
    nc = bacc.Bacc("TRN2", target_bir_lowering=False, debug=False,
                   num_devices=N_CORES)
    x_ap = nc.dram_tensor("x", [b_core, 3, 512, 512], F32R, kind="ExternalInput").ap()
    y_ap = nc.dram_tensor("y", [b_core, 3, 512, 512], F32, kind="ExternalOutput").ap()
    stats_ap = nc.dram_tensor("stats", [128, 48], F32, kind="ExternalOutput").ap()

    cdefs = {
        "rhs1": ([3, 128, 256], F32R),
        "w1h": ([128, 128], F32R),
        "we": ([128, 64], F32R),
        "w2h": ([128, 128], BF16),
        "wfh": ([64, 128], BF16),
        "w2w": ([128, 128], F32R),
        "wfw": ([4, 128, 128], F32R),
        "rq": ([128, 512], F32),
        "q4": ([128, 2048], F32),
        "cy4": ([128, 2048], F32),
    }
    cdram = {k: nc.dram_tensor(k, shp, dt, kind="ExternalInput").ap()
             for k, (shp, dt) in cdefs.items()}

    with tile.TileContext(nc) as tc:
        with (
            tc.tile_pool(name="consts", bufs=1) as cpool,
            tc.tile_pool(name="io", bufs=2) as iopool,
            tc.tile_pool(name="mid", bufs=1) as mpool,
            tc.tile_pool(name="psum", bufs=1, space="PSUM") as pspool,
        ):
            # ---- constants into SBUF
            rhs1_t = cpool.tile([128, 3, 256], F32R, name="rhs1_t")
            nc.sync.dma_start(rhs1_t[:], cdram["rhs1"].rearrange("c p n -> p c n"))
            w1h_t = cpool.tile([128, 128], F32R, name="w1h_t")
            nc.sync.dma_start(w1h_t[:], cdram["w1h"])
            we_t = cpool.tile([128, 64], F32R, name="we_t")
            nc.sync.dma_start(we_t[:], cdram["we"])
            w2h_t = cpool.tile([128, 128], BF16, name="w2h_t")
            nc.sync.dma_start(w2h_t[:], cdram["w2h"])
            wfh_t = cpool.tile([64, 128], BF16, name="wfh_t")
            nc.sync.dma_start(wfh_t[:], cdram["wfh"])
            w2w_t = cpool.tile([128, 128], F32R, name="w2w_t")
            nc.sync.dma_start(w2w_t[:], cdram["w2w"])
            wfw_t = cpool.tile([128, 4, 128], F32R, name="wfw_t")
            nc.sync.dma_start(wfw_t[:], cdram["wfw"].rearrange("k p n -> p k n"))
            rq_t = cpool.tile([128, 512], F32, name="rq_t")
            nc.sync.dma_start(rq_t[:], cdram["rq"])
            q4_t = cpool.tile([128, 2048], F32, name="q4_t")
            nc.sync.dma_start(q4_t[:], cdram["q4"])
            cy4_t = cpool.tile([128, 2048], F32, name="cy4_t")
            nc.sync.dma_start(cy4_t[:], cdram["cy4"])

            stats_t = cpool.tile([128, 48], F32, name="stats_t")
            nc.vector.memset(stats_t[:], 0.0)
            zscr = cpool.tile([64, 1024], BF16, name="zscr")

            WFW_CB_G, WFW_CB_B, WFW_CR_R, WFW_CR_G = 0, 1, 2, 3

            for b in range(b_core):
                in_t = iopool.tile([128, 3, 4, 512], F32R, tag="in_t", name=f"in_{b}")
                nc.sync.dma_start(
                    in_t[:], x_ap[b].rearrange("c (wc p) h -> p c wc h", p=128))

                m1 = mpool.tile([128, 4, 4, 256], F32R, tag="m1", name=f"m1_{b}")
                qy = mpool.tile([128, 4, 512], F32, tag="qy", name=f"qy_{b}")
                qc = mpool.tile([64, 4, 512], F32, tag="qc", name=f"qc_{b}")

                for hc in range(4):
                    # ---- pass1: contract w (+color, +chroma w-pool), per 2 squares
                    for wcp in range(2):
                        ps1 = pspool.tile([128, 512], F32, tag="p1", bufs=2,
                                          name=f"ps1_{b}_{hc}_{wcp}")
                        n_mm = 0
                        for wi in range(2):
                            wc = wcp * 2 + wi
                            for c in range(3):
                                nc.tensor.matmul(
                                    ps1[:, wi * 256:(wi + 1) * 256],
                                    lhsT=in_t[:, c, wc,
                                              hc * 128:(hc + 1) * 128],
                                    rhs=rhs1_t[:, c, :],
                                    start=(n_mm == 0), stop=(n_mm == 5),
                                )
                                n_mm += 1
                        nc.scalar.copy(m1[:, hc, 2 * wcp:2 * wcp + 2, :], ps1[:])

                    # ---- pass2: contract h -> DCT coeffs; evict fused with *rq
                    ps2y = pspool.tile([128, 512], F32, tag="p2y", bufs=1,
                                       name=f"ps2y_{b}_{hc}")
                    nc.tensor.matmul(
                        ps2y[:],
                        lhsT=w1h_t[:],
                        rhs=m1[:, hc, :, 0:128],
                        start=True, stop=True,
                    )
                    nc.vector.tensor_mul(qy[:, hc, :], ps2y[:], rq_t[:])

                    ps2c = pspool.tile([64, 512], F32, tag="p2c", bufs=2,
                                       name=f"ps2c_{b}_{hc}")
                    cview = m1[:, hc, :, 128:256].rearrange(
                        "p wc (ch u) -> p ch wc u", ch=2)
                    nc.tensor.matmul(
                        ps2c[:],
                        lhsT=we_t[:],
                        rhs=cview,
                        start=True, stop=True,
                    )
                    nc.vector.tensor_mul(qc[:, hc, :], ps2c[:], rq_t[0:64, :])

                # ---- middle: quantize (round-to-nearest-even via magic), stats
                qyf = qy.rearrange("p a n -> p (a n)")
                qcf = qc.rearrange("p a n -> p (a n)")
                nc.gpsimd.tensor_sub(qyf, qyf, cy4_t[:])
                nc.gpsimd.tensor_scalar(qyf, qyf, MAGIC, MAGIC,
                                        op0=ALU.add, op1=ALU.subtract)
                nc.gpsimd.tensor_scalar(qcf, qcf, MAGIC, MAGIC,
                                        op0=ALU.add, op1=ALU.subtract)

                nc.vector.reduce_sum(stats_t[:, b:b + 1], qyf, axis=AX.X,
                                     apply_absolute_value=True)
                qc4 = qc.rearrange("p a (ch u) -> p a ch u", ch=2)
                nc.vector.reduce_sum(stats_t[0:64, 4 + 8 * b:12 + 8 * b], qc4,
                                     axis=AX.X, apply_absolute_value=True)
                nc.vector.tensor_scalar(zscr[:, :].rearrange("p (a u) -> p a u", a=4),
                                        qc4[:, :, 1, :], 0.0, 0.0,
                                        op0=ALU.is_equal, op1=ALU.add,
                                        accum_out=stats_t[0:64, 36 + b:37 + b])

                dy = mpool.tile([128, 4, 512], BF16, tag="dy", name=f"dy_{b}")
                dc_ = mpool.tile([64, 4, 512], BF16, tag="dc", name=f"dc_{b}")
                nc.gpsimd.tensor_mul(dy.rearrange("p a n -> p (a n)"), qyf, q4_t[:])
                nc.gpsimd.tensor_mul(dc_.rearrange("p a n -> p (a n)"), qcf,
                                     q4_t[0:64, :])

                # ---- pass3: IDCT contract h' (+chroma h-upsample), bf16
                m3y = mpool.tile([128, 4, 512], F32R, tag="m3y", name=f"m3y_{b}")
                for j in range(4):
                    ps3 = pspool.tile([128, 512], F32, tag="p3", bufs=1,
                                      name=f"ps3y_{b}_{j}")
                    for i in range(4):
                        nc.tensor.matmul(
                            ps3[:, i * 128:(i + 1) * 128],
                            lhsT=dy[:, i, j * 128:(j + 1) * 128],
                            rhs=w2h_t[:],
                            start=(i == 0), stop=(i == 3),
                        )
                    nc.vector.tensor_copy(m3y[:, j, :], ps3[:])

                m3c = mpool.tile([128, 2, 2, 512], F32R, tag="m3c", name=f"m3c_{b}")
                for ch in range(2):
                    for s in range(2):
                        ps3c = pspool.tile([128, 512], F32, tag="p3", bufs=1,
                                           name=f"ps3c_{b}_{ch}_{s}")
                        for k in range(4):
                            nc.tensor.matmul(
                                ps3c[:, k * 128:(k + 1) * 128],
                                lhsT=dc_[:, k,
                                         ch * 256 + s * 128:ch * 256 + (s + 1) * 128],
                                rhs=wfh_t[:],
                                start=(k == 0), stop=(k == 3),
                            )
                        nc.vector.tensor_copy(m3c[:, ch, s, :], ps3c[:])

                # ---- pass4: IDCT contract w' (+chroma w-up) + XF2 color, +HALF
                out_t = iopool.tile([128, 3, 4, 512], F32, tag="out_t",
                                    name=f"out_{b}")
                for wc in range(4):
                    s, hv = wc // 2, (wc % 2) * 64
                    rhs_y = m3y[:, wc, :]
                    rhs_cb = m3c[hv:hv + 64, 0, s, :]
                    rhs_cr = m3c[hv:hv + 64, 1, s, :]
                    wslice = slice(hv, hv + 64)

                    ps4 = pspool.tile([128, 512], F32, tag="p4", bufs=2,
                                      name=f"ps4r_{b}_{wc}")
                    nc.tensor.matmul(ps4[:], lhsT=w2w_t[:], rhs=rhs_y,
                                     start=True, stop=False)
                    nc.tensor.matmul(ps4[:], lhsT=wfw_t[wslice, WFW_CR_R, :],
                                     rhs=rhs_cr, start=False, stop=True)
                    nc.scalar.activation(out_t[:, 0, wc, :], ps4[:], ACTF.Copy,
                                         bias=HALF)

                    ps4g = pspool.tile([128, 512], F32, tag="p4", bufs=2,
                                       name=f"ps4g_{b}_{wc}")
                    nc.tensor.matmul(ps4g[:], lhsT=w2w_t[:], rhs=rhs_y,
                                     start=True, stop=False)
                    nc.tensor.matmul(ps4g[:], lhsT=wfw_t[wslice, WFW_CB_G, :],
                                     rhs=rhs_cb, start=False, stop=False)
                    nc.tensor.matmul(ps4g[:], lhsT=wfw_t[wslice, WFW_CR_G, :],
                                     rhs=rhs_cr, start=False, stop=True)
                    nc.scalar.activation(out_t[:, 1, wc, :], ps4g[:], ACTF.Copy,
                                         bias=HALF)

                    ps4b = pspool.tile([128, 512], F32, tag="p4", bufs=2,
                                       name=f"ps4b_{b}_{wc}")
                    nc.tensor.matmul(ps4b[:], lhsT=w2w_t[:], rhs=rhs_y,
                                     start=True, stop=False)
                    nc.tensor.matmul(ps4b[:], lhsT=wfw_t[wslice, WFW_CB_B, :],
                                     rhs=rhs_cb, start=False, stop=True)
                    nc.scalar.activation(out_t[:, 2, wc, :], ps4b[:], ACTF.Copy,
                                         bias=HALF)

                nc.sync.dma_start(
                    y_ap[b].rearrange("c (wc p) h -> p c wc h", p=128), out_t[:])

            nc.sync.dma_start(stats_ap[:], stats_t[:])

    nc.compile()
    return nc


_CACHED_NC = None


def _get_nc():
    global _CACHED_NC
    if _CACHED_NC is None:
        _CACHED_NC = build_nc()
    return _CACHED_NC


def make_in_maps(x: np.ndarray, qtable: np.ndarray) -> list[dict[str, np.ndarray]]:
    consts = make_consts(qtable)
    in_maps = []
    for c in range(N_CORES):
        m = {"x": np.ascontiguousarray(x[c * B_CORE:(c + 1) * B_CORE])}
        m.update(consts)
        in_maps.append(m)
    return in_maps


def assemble(results: list[dict[str, np.ndarray]]):
    y = np.concatenate([r["y"] for r in results], axis=0)
    ny = 32 * 512 * 512
    nc_ = 32 * 256 * 256
    ay = acb = acr = zc = 0.0
    for r in results:
        st = r["stats"]
        ay += float(st[:, 0:4].sum())
        cs = st[0:64, 4:36].reshape(64, 4, 4, 2)     # [p, b, hc, ch]
        acb += float(cs[..., 0].sum())
        acr += float(cs[..., 1].sum())
        zc += float(st[0:64, 36:40].sum())
    means = np.float32(ay / ny + acb / nc_ + acr / nc_)
    nzeros = np.float32(zc / nc_)
    return y, means, nzeros


def kernel(x: np.ndarray, qtable: np.ndarray):
    x = np.asarray(x, dtype=np.float32)
    qtable = np.asarray(qtable, dtype=np.float32)
    nc = _get_nc()
    res = run_bass_kernel_spmd(nc, make_in_maps(x, qtable),
                               core_ids=list(range(N_CORES)))
    return assemble(res.results)


# revision 6
# speedup vs baseline: 523.4685x; 523.4685x over previous
"""JPEG layer (DCT quantize/dequantize roundtrip) as a Bass/Tile kernel on 8 trn2 cores.

Data parallel: 32 images -> 4 per core. Everything per-image stays in SBUF.

Pipeline per image (channel planes 512x512 f32):
  DMA in [128(wlow), c, wc, h]
  pass1 (PE, f32r, data-as-weights): contract w within 8(16)-blocks, fused color
        transform (XF1) + chroma 2x2 w-pooling. out M1[h-part, (wc,[Y|Cb|Cr])]
  pass2 (PE, f32r, const weights):   contract h. out DCT coeffs [h'-part, w']
  middle (DVE/GPSIMD/ACT): quant = rne((dct - dcshift)/q) via magic-number trick,
        stats (mean|quant|, zero count), decomp = quant*q -> bf16
  pass3 (PE, bf16, data-as-weights): IDCT contract h' (+chroma h-upsample),
        out M3[w'-part, h]
  pass4 (PE, f32r, const weights):   IDCT contract w' (+chroma w-upsample),
        fused XF2 color transform accumulated in PSUM; +HALF on eviction
  DMA out
"""

import math

import ml_dtypes
import numpy as np

import concourse.bass as bass
import concourse.mybir as mybir
import concourse.tile as tile
from concourse import bacc
from concourse.bass_utils import run_bass_kernel_spmd

N_CORES = 8
B_CORE = 4          # images per core
BS = 8
HALF = 128.0 / 255.0
MAGIC = float(1.5 * 2 ** 23)

F32 = mybir.dt.float32
F32R = mybir.dt.float32r
BF16 = mybir.dt.bfloat16
AX = mybir.AxisListType
ALU = mybir.AluOpType
ACTF = mybir.ActivationFunctionType

XF1 = np.array([[0.299, 0.587, 0.114],
                [-0.168735892, -0.331264108, 0.5],
                [0.5, -0.418687589, -0.081312411]], dtype=np.float32)
XF2 = np.array([[1.0, 0.0, 1.402],
                [1.0, -0.344136286, -0.714136286],
                [1.0, 1.772, 0.0]], dtype=np.float32)


def _dctmtx():
    j = np.arange(BS)
    i = np.arange(BS)
    m = np.sqrt(2.0 / BS) * np.cos(np.pi * (2 * j[None, :] + 1) * i[:, None] / (2 * BS))
    m[0, :] = 1.0 / np.sqrt(BS)
    return m.astype(np.float32)


def _blockdiag(block, nblk):
    """block [r, c] -> [nblk*r, nblk*c] block diagonal."""
    r, c = block.shape
    out = np.zeros((nblk * r, nblk * c), dtype=np.float32)
    for b in range(nblk):
        out[b * r:(b + 1) * r, b * c:(b + 1) * c] = block
    return out


def make_consts(qtable: np.ndarray) -> dict[str, np.ndarray]:
    D = _dctmtx()
    # E: pooled DCT row op (8x16), F: IDCT+upsample row op (16x8)
    E = np.zeros((8, 16), dtype=np.float32)
    for u in range(8):
        for j in range(16):
            E[u, j] = 0.5 * D[u, j // 2]
    F = np.zeros((16, 8), dtype=np.float32)
    for r in range(16):
        for v in range(8):
            F[r, v] = D[v, r // 2]

    WY = _blockdiag(D.T, 16)            # [128,128]  w-DCT, used transposed-contract
    WE = _blockdiag(E.T, 8)             # [128,64]   pooled w(or h)-DCT
    rhs1 = np.stack([
        np.concatenate([WY * XF1[0, c], WE * XF1[1, c], WE * XF1[2, c]], axis=1)
        for c in range(3)
    ])                                   # [3,128,256]
    W2h = _blockdiag(D, 16)             # [128,128]  IDCT contract h' (rhs pass3 Y)
    WFh = _blockdiag(F.T, 8)            # [64,128]   IDCT+up contract h'_c
    W2w = _blockdiag(D, 16)             # [128,128]  IDCT contract w' (lhsT pass4 Y)
    WFw64 = _blockdiag(F.T, 8)          # [64,128]
    WFw = np.concatenate([WFw64, WFw64], axis=0)    # [128,128] dup for base 0/64
    wfw = np.stack([
        WFw * XF2[1, 1],   # cb->G
        WFw * XF2[2, 1],   # cb->B
        WFw * XF2[0, 2],   # cr->R
        WFw * XF2[1, 2],   # cr->G
    ])                                   # [4,128,128]

    q = qtable.astype(np.float32)        # q[u, v]
    p = np.arange(128)
    f = np.arange(512)
    qpf = q[np.ix_(f % 8, p % 8)].T      # [128,512]: q[f%8, p%8]
    rq = (1.0 / qpf).astype(np.float32)
    q4 = np.tile(qpf, (1, 4)).astype(np.float32)          # [128,2048]
    cy = np.zeros((128, 512), dtype=np.float32)
    dc = np.ix_(p % 8 == 0, f % 8 == 0)
    cy[dc] = (8.0 * HALF) * rq[dc]
    cy4 = np.tile(cy, (1, 4)).astype(np.float32)          # [128,2048]

    return {
        "rhs1": rhs1,
        "w1h": WY,                       # lhsT pass2 Y (same matrix as WY)
        "we": WE,
        "w2h": W2h.astype(ml_dtypes.bfloat16),
        "wfh": WFh.astype(ml_dtypes.bfloat16),
        "w2w": W2w,
        "wfw": wfw,
        "rq": rq,
        "q4": q4,
        "cy4": cy4,
    }


def build_nc(b_core: int = B_CORE, repeat: int = 1):
    nc = bacc.Bacc("TRN2", target_bir_lowering=False, debug=False,
                   num_devices=N_CORES)
    x_ap = nc.dram_tensor("x", [b_core, 3, 512, 512], F32R, kind="ExternalInput").ap()
    y_ap = nc.dram_tensor("y", [b_core, 3, 512, 512], F32, kind="ExternalOutput").ap()
    stats_ap = nc.dram_tensor("stats", [128, 48], F32, kind="ExternalOutput").ap()

    cdefs = {
        "rhs1": ([3, 128, 256], F32R),
        "w1h": ([128, 128], F32R),
        "we": ([128, 64], F32R),
        "w2h": ([128, 128], BF16),
        "wfh": ([64, 128], BF16),
        "w2w": ([128, 128], F32R),
        "wfw": ([4, 128, 128], F32R),
        "rq": ([128, 512], F32),
        "q4": ([128, 2048], F32),
        "cy4": ([128, 2048], F32),
    }
    cdram = {k: nc.dram_tensor(k, shp, dt, kind="ExternalInput").ap()
             for k, (shp, dt) in cdefs.items()}

    with tile.TileContext(nc) as tc:
        with (
            tc.tile_pool(name="consts", bufs=1) as cpool,
            tc.tile_pool(name="io", bufs=2) as iopool,
            tc.tile_pool(name="mid", bufs=1) as mpool,
            tc.tile_pool(name="psum", bufs=1, space="PSUM") as pspool,
        ):
            # ---- constants into SBUF
            rhs1_t = cpool.tile([128, 3, 256], F32R, name="rhs1_t")
            nc.sync.dma_start(rhs1_t[:], cdram["rhs1"].rearrange("c p n -> p c n"))
            w1h_t = cpool.tile([128, 128], F32R, name="w1h_t")
            nc.sync.dma_start(w1h_t[:], cdram["w1h"])
            we_t = cpool.tile([128, 64], F32R, name="we_t")
            nc.sync.dma_start(we_t[:], cdram["we"])
            w2h_t = cpool.tile([128, 128], BF16, name="w2h_t")
            nc.sync.dma_start(w2h_t[:], cdram["w2h"])
            wfh_t = cpool.tile([64, 128], BF16, name="wfh_t")
            nc.sync.dma_start(wfh_t[:], cdram["wfh"])
            w2w_t = cpool.tile([128, 128], F32R, name="w2w_t")
            nc.sync.dma_start(w2w_t[:], cdram["w2w"])
            wfw_t = cpool.tile([128, 4, 128], F32R, name="wfw_t")
            nc.sync.dma_start(wfw_t[:], cdram["wfw"].rearrange("k p n -> p k n"))
            rq_t = cpool.tile([128, 512], F32, name="rq_t")
            nc.sync.dma_start(rq_t[:], cdram["rq"])
            q4_t = cpool.tile([128, 2048], F32, name="q4_t")
            nc.sync.dma_start(q4_t[:], cdram["q4"])
            cy4_t = cpool.tile([128, 2048], F32, name="cy4_t")
            nc.sync.dma_start(cy4_t[:], cdram["cy4"])

            stats_t = cpool.tile([128, 48], F32, name="stats_t")
            nc.vector.memset(stats_t[:], 0.0)
            zscr = cpool.tile([64, 1024], BF16, name="zscr")

            WFW_CB_G, WFW_CB_B, WFW_CR_R, WFW_CR_G = 0, 1, 2, 3

            for rep in range(repeat):
              for b in range(b_core):
                in_t = iopool.tile([128, 3, 4, 512], F32R, tag="in_t",
                                   name=f"in_{rep}_{b}")
                nc.sync.dma_start(
                    in_t[:], x_ap[b].rearrange("c (wc p) h -> p c wc h", p=128))

                m1 = mpool.tile([128, 4, 4, 256], F32R, tag="m1", name=f"m1_{b}")
                qy = mpool.tile([128, 4, 512], F32, tag="qy", name=f"qy_{b}")
                qc = mpool.tile([64, 4, 512], F32, tag="qc", name=f"qc_{b}")

                for hc in range(4):
                    # ---- pass1: contract w (+color, +chroma w-pool), per 2 squares
                    for wcp in range(2):
                        ps1 = pspool.tile([128, 512], F32, tag="p1", bufs=2,
                                          name=f"ps1_{b}_{hc}_{wcp}")
                        n_mm = 0
                        for wi in range(2):
                            wc = wcp * 2 + wi
                            for c in range(3):
                                nc.tensor.matmul(
                                    ps1[:, wi * 256:(wi + 1) * 256],
                                    lhsT=in_t[:, c, wc,
                                              hc * 128:(hc + 1) * 128],
                                    rhs=rhs1_t[:, c, :],
                                    start=(n_mm == 0), stop=(n_mm == 5),
                                )
                                n_mm += 1
                        nc.scalar.copy(m1[:, hc, 2 * wcp:2 * wcp + 2, :], ps1[:])

                    # ---- pass2: contract h -> DCT coeffs; evict fused with *rq
                    ps2y = pspool.tile([128, 512], F32, tag="p2y", bufs=1,
                                       name=f"ps2y_{b}_{hc}")
                    nc.tensor.matmul(
                        ps2y[:],
                        lhsT=w1h_t[:],
                        rhs=m1[:, hc, :, 0:128],
                        start=True, stop=True,
                    )
                    nc.vector.tensor_mul(qy[:, hc, :], ps2y[:], rq_t[:])

                    ps2c = pspool.tile([64, 512], F32, tag="p2c", bufs=2,
                                       name=f"ps2c_{b}_{hc}")
                    cview = m1[:, hc, :, 128:256].rearrange(
                        "p wc (ch u) -> p ch wc u", ch=2)
                    nc.tensor.matmul(
                        ps2c[:],
                        lhsT=we_t[:],
                        rhs=cview,
                        start=True, stop=True,
                    )
                    nc.vector.tensor_mul(qc[:, hc, :], ps2c[:], rq_t[0:64, :])

                # ---- middle: quantize (round-to-nearest-even via magic), stats
                qyf = qy.rearrange("p a n -> p (a n)")
                qcf = qc.rearrange("p a n -> p (a n)")
                nc.gpsimd.tensor_sub(qyf, qyf, cy4_t[:])
                nc.gpsimd.tensor_scalar(qyf, qyf, MAGIC, MAGIC,
                                        op0=ALU.add, op1=ALU.subtract)
                nc.gpsimd.tensor_scalar(qcf, qcf, MAGIC, MAGIC,
                                        op0=ALU.add, op1=ALU.subtract)

                nc.vector.reduce_sum(stats_t[:, b:b + 1], qyf, axis=AX.X,
                                     apply_absolute_value=True)
                qc4 = qc.rearrange("p a (ch u) -> p a ch u", ch=2)
                nc.vector.reduce_sum(stats_t[0:64, 4 + 8 * b:12 + 8 * b], qc4,
                                     axis=AX.X, apply_absolute_value=True)
                nc.vector.tensor_scalar(zscr[:, :].rearrange("p (a u) -> p a u", a=4),
                                        qc4[:, :, 1, :], 0.0, 0.0,
                                        op0=ALU.is_equal, op1=ALU.add,
                                        accum_out=stats_t[0:64, 36 + b:37 + b])

                dy = mpool.tile([128, 4, 512], BF16, tag="dy", name=f"dy_{b}")
                dc_ = mpool.tile([64, 4, 512], BF16, tag="dc", name=f"dc_{b}")
                nc.gpsimd.tensor_mul(dy.rearrange("p a n -> p (a n)"), qyf, q4_t[:])
                nc.gpsimd.tensor_mul(dc_.rearrange("p a n -> p (a n)"), qcf,
                                     q4_t[0:64, :])

                # ---- pass3: IDCT contract h' (+chroma h-upsample), bf16
                m3y = mpool.tile([128, 4, 512], F32R, tag="m3y", name=f"m3y_{b}")
                for j in range(4):
                    ps3 = pspool.tile([128, 512], F32, tag="p3", bufs=1,
                                      name=f"ps3y_{b}_{j}")
                    for i in range(4):
                        nc.tensor.matmul(
                            ps3[:, i * 128:(i + 1) * 128],
                            lhsT=dy[:, i, j * 128:(j + 1) * 128],
                            rhs=w2h_t[:],
                            start=(i == 0), stop=(i == 3),
                        )
                    nc.vector.tensor_copy(m3y[:, j, :], ps3[:])

                m3c = mpool.tile([128, 2, 2, 512], F32R, tag="m3c", name=f"m3c_{b}")
                for ch in range(2):
                    for s in range(2):
                        ps3c = pspool.tile([128, 512], F32, tag="p3", bufs=1,
                                           name=f"ps3c_{b}_{ch}_{s}")
                        for k in range(4):
                            nc.tensor.matmul(
                                ps3c[:, k * 128:(k + 1) * 128],
                                lhsT=dc_[:, k,
                                         ch * 256 + s * 128:ch * 256 + (s + 1) * 128],
                                rhs=wfh_t[:],
                                start=(k == 0), stop=(k == 3),
                            )
                        nc.vector.tensor_copy(m3c[:, ch, s, :], ps3c[:])

                # ---- pass4: IDCT contract w' (+chroma w-up) + XF2 color, +HALF
                out_t = iopool.tile([128, 3, 4, 512], F32, tag="out_t",
                                    name=f"out_{b}")
                for wc in range(4):
                    s, hv = wc // 2, (wc % 2) * 64
                    rhs_y = m3y[:, wc, :]
                    rhs_cb = m3c[hv:hv + 64, 0, s, :]
                    rhs_cr = m3c[hv:hv + 64, 1, s, :]
                    wslice = slice(hv, hv + 64)

                    ps4 = pspool.tile([128, 512], F32, tag="p4", bufs=2,
                                      name=f"ps4r_{b}_{wc}")
                    nc.tensor.matmul(ps4[:], lhsT=w2w_t[:], rhs=rhs_y,
                                     start=True, stop=False)
                    nc.tensor.matmul(ps4[:], lhsT=wfw_t[wslice, WFW_CR_R, :],
                                     rhs=rhs_cr, start=False, stop=True)
                    nc.scalar.activation(out_t[:, 0, wc, :], ps4[:], ACTF.Copy,
                                         bias=HALF)

                    ps4g = pspool.tile([128, 512], F32, tag="p4", bufs=2,
                                       name=f"ps4g_{b}_{wc}")
                    nc.tensor.matmul(ps4g[:], lhsT=w2w_t[:], rhs=rhs_y,
                                     start=True, stop=False)
                    nc.tensor.matmul(ps4g[:], lhsT=wfw_t[wslice, WFW_CB_G, :],
                                     rhs=rhs_cb, start=False, stop=False)
                    nc.tensor.matmul(ps4g[:], lhsT=wfw_t[wslice, WFW_CR_G, :],
                                     rhs=rhs_cr, start=False, stop=True)
                    nc.scalar.activation(out_t[:, 1, wc, :], ps4g[:], ACTF.Copy,
                                         bias=HALF)

                    ps4b = pspool.tile([128, 512], F32, tag="p4", bufs=2,
                                       name=f"ps4b_{b}_{wc}")
                    nc.tensor.matmul(ps4b[:], lhsT=w2w_t[:], rhs=rhs_y,
                                     start=True, stop=False)
                    nc.tensor.matmul(ps4b[:], lhsT=wfw_t[wslice, WFW_CB_B, :],
                                     rhs=rhs_cb, start=False, stop=True)
                    nc.scalar.activation(out_t[:, 2, wc, :], ps4b[:], ACTF.Copy,
                                         bias=HALF)

                nc.sync.dma_start(
                    y_ap[b].rearrange("c (wc p) h -> p c wc h", p=128), out_t[:])

            nc.sync.dma_start(stats_ap[:], stats_t[:])

    nc.compile()
    return nc


_CACHED_NC = None


def _get_nc():
    global _CACHED_NC
    if _CACHED_NC is None:
        _CACHED_NC = build_nc()
    return _CACHED_NC


def make_in_maps(x: np.ndarray, qtable: np.ndarray) -> list[dict[str, np.ndarray]]:
    consts = make_consts(qtable)
    in_maps = []
    for c in range(N_CORES):
        m = {"x": np.ascontiguousarray(x[c * B_CORE:(c + 1) * B_CORE])}
        m.update(consts)
        in_maps.append(m)
    return in_maps


def assemble(results: list[dict[str, np.ndarray]]):
    y = np.concatenate([r["y"] for r in results], axis=0)
    ny = 32 * 512 * 512
    nc_ = 32 * 256 * 256
    ay = acb = acr = zc = 0.0
    for r in results:
        st = r["stats"]
        ay += float(st[:, 0:4].sum())
        cs = st[0:64, 4:36].reshape(64, 4, 4, 2)     # [p, b, hc, ch]
        acb += float(cs[..., 0].sum())
        acr += float(cs[..., 1].sum())
        zc += float(st[0:64, 36:40].sum())
    means = np.float32(ay / ny + acb / nc_ + acr / nc_)
    nzeros = np.float32(zc / nc_)
    return y, means, nzeros


def kernel(x: np.ndarray, qtable: np.ndarray):
    x = np.asarray(x, dtype=np.float32)
    qtable = np.asarray(qtable, dtype=np.float32)
    nc = _get_nc()
    res = run_bass_kernel_spmd(nc, make_in_maps(x, qtable),
                               core_ids=list(range(N_CORES)))
    return assemble(res.results)
